# revision 51
# baseline (speedup 1.0000x reference)
"""Trainium2 Bass kernel for CapsuleParall dynamic routing (v2).

Math (per (b, n) pair, u_hat[i,o] = u[i] * W[n][i,o]):
    s_1[o] = sum_i u_hat[i,o] * c0[i,o]
    v_t    = squash(s_t + bias)          (squash over o)
    V_t    = v_1 + ... + v_t
    c_t[i,o] = softmax_o(u_hat[i,o] * V_t[o])
    s_{t+1}[o] = sum_i u_hat[i,o] * c_t[i,o]
    out    = squash(s_R + bias)

|tt| = |u_hat * V| <= ~0.1, so exp is replaced by a 2nd-order Taylor
expansion; with M1[i] = sum_o W[i,o] V[o] and Z ~= OF + u*M1:

    s = (1/OF) * [ sum_i W[i,o] (u - (u^2/OF) M1)[i] ]            (A chain)
        + V[o] * [ sum_i W2[i,o] ((u^2/OF) - (u^3/OF^2) M1)[i] ]  (B chain)

Both chains are LINEAR in M1, and M1 itself is linear in the per-round
squash outputs, so the A/B PSUM regions are accumulated ONCE at phase 1
(u / u^2 movings) and then only receive the per-iteration delta movings
dm1 = -(u^2/OF) * dM1 and dm2 = -(u^3/OF^2) * dM1 where
dM1 = coef_t * (W^T @ sb_t).  No M1 state tensor is ever materialized.

Sharding: num(16) split across 8 cores (2 capsules/core, all 32
batches), so every PE stationary W[n,j] serves 32 moving columns
instead of 4 — 8x fewer Ldweights/matmul instructions and 8x less
weight DMA than batch-sharding.  Host pre-packs W and W^2 in the exact
SBUF layout (partition p holds rows i = p*9..p*9+8) as bf16; W^T is
built on-device with PE transposes.  Pair columns are n-major
(pair = n*32 + b).
"""

import sys

sys.path.insert(0, "/opt/trn_rl_repo")

from contextlib import ExitStack

import numpy as np
import ml_dtypes

import concourse.bass as bass
import concourse.bacc as bacc
import concourse.mybir as mybir
import concourse.tile as tile
from concourse import masks
from concourse.bass_utils import run_bass_kernel_spmd

F32 = mybir.dt.float32
BF16 = mybir.dt.bfloat16
N_CORES = 8
DEFAULT_KEY = (2, 32, 1152, 128, 3, 1.0, True)  # NC_N, B, IN_F, OUT_F, R, kscale, uniform


def _build(NC_N, B, IN_F, OUT_F, routings, kscale, uniform_c0):
    """Per-core module. NC_N capsules, all B batches. kscale = c00*OF."""
    P = 128
    assert IN_F % P == 0 and OUT_F == P
    J = IN_F // P                      # 9 slots; i = p*J + j
    PAIRS = B * NC_N                   # 64 pairs, n-major: pair = n*B + b
    mult = mybir.AluOpType.mult
    add = mybir.AluOpType.add
    rOF = 1.0 / OUT_F

    nc = bacc.Bacc("TRN2", target_bir_lowering=False, debug=False)

    u_dram = nc.dram_tensor("u", [PAIRS, IN_F], BF16, kind="ExternalInput")
    w_dram = nc.dram_tensor("wbf", [P, NC_N * J * OUT_F], BF16, kind="ExternalInput")
    w2_dram = nc.dram_tensor("w2bf", [P, NC_N * J * OUT_F], BF16, kind="ExternalInput")
    b_dram = nc.dram_tensor("biasc", [P, NC_N], F32, kind="ExternalInput")
    br_dram = nc.dram_tensor("biasr", [1, NC_N * OUT_F], F32, kind="ExternalInput")
    if not uniform_c0:
        wc_dram = nc.dram_tensor("wcbf", [P, NC_N * J * OUT_F], BF16,
                                 kind="ExternalInput")
    out_dram = nc.dram_tensor("out", [B, NC_N, OUT_F], F32, kind="ExternalOutput")

    with tile.TileContext(nc) as tc, ExitStack() as ctx:
        const = ctx.enter_context(tc.tile_pool(name="const", bufs=1))
        state = ctx.enter_context(tc.tile_pool(name="state", bufs=1))
        sq_pool = ctx.enter_context(tc.tile_pool(name="sq", bufs=3))
        # PSUM banks: ptr 2, pS 1 (all four A/B accumulation chains live in
        # ONE bank as a single never-stopped group; start=True pending-zeroes
        # the whole 2KB region), pM 2x1, paux 2x1  -> 7 of 8.
        ptr = ctx.enter_context(
            tc.tile_pool(name="ptr", bufs=3, space=bass.MemorySpace.PSUM))
        pS = ctx.enter_context(
            tc.tile_pool(name="pS", bufs=1, space=bass.MemorySpace.PSUM))
        pM = ctx.enter_context(
            tc.tile_pool(name="pM", bufs=2, space=bass.MemorySpace.PSUM))
        paux = ctx.enter_context(
            tc.tile_pool(name="paux", bufs=1, space=bass.MemorySpace.PSUM))
        pD = ctx.enter_context(
            tc.tile_pool(name="pD", bufs=1, space=bass.MemorySpace.PSUM))

        # ---- resident tensors ----
        W_sb = const.tile([P, NC_N, J, OUT_F], BF16)   # W[p*J+j + n*IN_F, o]
        W2_sb = const.tile([P, NC_N, J, OUT_F], BF16)  # W^2 (host-packed)
        WT_sb = const.tile([P, NC_N, J, P], BF16)      # W^T: [o, n, j, p]
        if not uniform_c0:
            Wc_sb = const.tile([P, NC_N, J, OUT_F], BF16)
        u_nat = const.tile([PAIRS, IN_F], BF16)        # b-major rows
        bias_c = const.tile([P, NC_N], F32)            # bias cols [o, n]
        bias_r = const.tile([1, NC_N * OUT_F], F32)    # bias rows on partition 0
        u_sb = const.tile([P, J, PAIRS], BF16)         # u cols, n-major pairs
        un2_128 = const.tile([P, J, PAIRS], BF16)      # -u^2/OF
        identf = const.tile([P, P], F32)
        identb = const.tile([P, P], BF16)
        ones_col = const.tile([P, 1], F32)
        ones_row = const.tile([1, P], F32)
        ones_colb = const.tile([P, 1], BF16)
        ones_rowb = const.tile([1, P], BF16)
        neg1_col = const.tile([P, 1], F32)

        V = state.tile([P, PAIRS], BF16)               # cumulative v
        u128 = state.tile([P, J, PAIRS], BF16)         # u/OF
        u2b = state.tile([P, J, PAIRS], BF16)
        dm1 = state.tile([P, J, PAIRS], BF16)          # -u^2/OF * (W^T@sb)
        dm2 = state.tile([P, J, PAIRS], BF16)          # -u^3/OF^2 * (W^T@sb)
        sb0 = state.tile([P, PAIRS], BF16)             # rOF*A0 (+bias)
        pacc = state.tile([P, PAIRS], BF16)            # sb0 + rOF*sum cb*D_A
        qacc = state.tile([P, PAIRS], BF16)            # B0 + sum cb*D_B
        vrows = state.tile([PAIRS, OUT_F], F32)
        srows = state.tile([PAIRS, OUT_F], F32)

        def cols(t3, j, n):
            # [P, J, PAIRS] -> [P, B] contiguous cols at slot j, capsule n
            ap = t3[:, :, :]
            return bass.AP(ap.tensor, ap.offset + j * PAIRS + n * B,
                           [ap.ap[0], [1, B]])

        def ncols(t2, n):
            # [P, PAIRS] -> [P, B] contiguous cols for capsule n
            ap = t2[:, :]
            return bass.AP(ap.tensor, ap.offset + n * B, [ap.ap[0], [1, B]])

        bias_ap = bias_c[:, :]
        bias_bc = bass.AP(bias_ap.tensor, bias_ap.offset,
                          [bias_ap.ap[0], [1, NC_N], [0, B]])

        # ---- input DMAs ----
        # u + W-n1 go through Pool SWDGE (short head, parallel with the
        # HWDGE gens); W-n0 via Act HWDGE; biases via SP ahead of W2 so the
        # tiny transfers beat W2 to the (serialized) DMA device.
        w_ap = w_dram.ap()

        def w_src(n):
            return bass.AP(w_ap.tensor, w_ap.offset + n * J * OUT_F,
                           [w_ap.ap[0], [1, J * OUT_F]])

        nc.gpsimd.dma_start(u_nat[:, :], u_dram.ap())
        nc.gpsimd.dma_start(W_sb[:, 1, :, :], w_src(1))
        nc.scalar.dma_start(W_sb[:, 0, :, :], w_src(0))
        nc.sync.dma_start(bias_c[:, :], b_dram.ap())
        nc.sync.dma_start(bias_r[:, :], br_dram.ap())
        nc.sync.dma_start(W2_sb[:, :, :, :], w2_dram.ap())
        if not uniform_c0:
            nc.sync.dma_start(Wc_sb[:, :, :, :], wc_dram.ap())

        zmov = const.tile([P, P], BF16)
        ofrow = const.tile([1, P], F32)
        nc.vector.memset(zmov[:, :], 0.0)
        masks.make_identity(nc, identb[:, :])
        nc.vector.memset(ones_col[:, :], 1.0)
        nc.vector.memset(ones_row[:, :], 1.0)
        nc.vector.memset(ones_colb[:, :], 1.0)
        nc.vector.memset(ones_rowb[:, :], 1.0)
        nc.vector.memset(neg1_col[:, :], -1.0)
        nc.vector.memset(ofrow[:, :], float(OUT_F))
        masks.make_identity(nc, identf[:, :])
        # warm all Act function tables used later, during the DMA window
        warm = state.tile([1, 4], F32)
        nc.scalar.activation(warm[:, 0:1], ones_col[0:1, 0:1],
                             mybir.ActivationFunctionType.Sqrt)
        nc.scalar.square(warm[:, 1:2], ones_col[0:1, 0:1])
        nc.scalar.copy(warm[:, 2:3], ones_col[0:1, 0:1])
        # PE p-state warmup: zero matmuls (no identity dependency) keep the
        # PE "busy" from t~0.3us so real matmuls run at full clock
        jp = ptr.tile([P, 8 * P], BF16, tag="tr")
        jpf = jp[:, :].bitcast(F32)
        for k in range(6):
            nc.tensor.matmul(jpf[0:1, 0:P], zmov[:, 0:1], zmov[:, :],
                             start=True, stop=True)

        # ---- u prep: batched transposes + big reorder copies ----
        u_ap = u_nat[:, :]
        for j0 in range(0, J, 8):
            jn = min(8, J - j0)
            tr = ptr.tile([P, 8 * P], BF16, tag="tr")
            for j in range(j0, j0 + jn):
                u_slice = bass.AP(u_ap.tensor, u_ap.offset + j,
                                  [u_ap.ap[0], [J, P]])
                nc.tensor.transpose(tr[0:P, (j - j0) * PAIRS:(j - j0 + 1) * PAIRS],
                                    u_slice, identb[:PAIRS, :PAIRS])
            # reorder b-major psum cols -> n-major u_sb cols, all jn at once
            tr_ap = tr[0:P, 0:jn * PAIRS]
            srcv = bass.AP(tr_ap.tensor, tr_ap.offset,
                           [tr_ap.ap[0], [PAIRS, jn], [1, NC_N], [NC_N, B]])
            dst_ap = u_sb[:, j0, :]
            dstv = bass.AP(dst_ap.tensor, dst_ap.offset,
                           [dst_ap.ap[0], [PAIRS, jn], [B, NC_N], [1, B]])
            nc.vector.tensor_copy(dstv, srcv)

        # ---- W^T transposes (PE) + psum->sbuf copies (n0->Act, n1->DVE) ----
        WT_ap = WT_sb[:, :, :, :]
        WT_flat = bass.AP(WT_ap.tensor, WT_ap.offset, [WT_ap.ap[0], [1, NC_N * J * P]])

        def wt_chunk(n):
            base = n * J * P
            for j0 in range(0, J, 8):
                jn = min(8, J - j0)
                trw = ptr.tile([P, 8 * P], BF16, tag="tr")
                for j in range(j0, j0 + jn):
                    nc.tensor.transpose(trw[:, (j - j0) * P:(j - j0 + 1) * P],
                                        W_sb[:, n, j, :], identb[:, :])
                dst = bass.AP(WT_flat.tensor, WT_flat.offset + base + j0 * P,
                              [WT_flat.ap[0], [1, jn * P]])
                if n == 0:
                    nc.vector.tensor_copy(dst, trw[:, :jn * P])
                else:
                    nc.scalar.copy(dst, trw[:, :jn * P])

        # ---- static psum regions (closed groups, read-only afterwards) ----
        # Sp[:, 0, n, :] = A0: sum W u (+ OF*bias folded in)
        # Sp[:, 1, n, :] = B0: sum W2 u^2/OF (Q1)
        # Sp[:, 2, n, :] = non-uniform phase-1 (Wc @ u)
        Sp = pS.tile([P, 3, NC_N, B], F32)
        fold_bias = (not uniform_c0) or abs(kscale - 1.0) < 1e-12

        # A chains (+ bias row) interleaved with W^T transposes on PE
        for n in range(NC_N):
            wt_chunk(n)
            for j in range(J):
                nc.tensor.matmul(Sp[:, 0, n, :], W_sb[:, n, j, :],
                                 cols(u_sb, j, n), start=(j == 0),
                                 stop=(j == J - 1 and not fold_bias))
            if fold_bias:
                nc.tensor.matmul(Sp[:, 0, n, :],
                                 bias_r[0:1, n * OUT_F:(n + 1) * OUT_F],
                                 ofrow[0:1, 0:B], start=False, stop=True)
        if not uniform_c0:
            for n in range(NC_N):
                for j in range(J):
                    nc.tensor.matmul(Sp[:, 2, n, :], Wc_sb[:, n, j, :],
                                     cols(u_sb, j, n), start=(j == 0),
                                     stop=(j == J - 1))

        def q1_chains():
            # moving is -u^2/OF so this region holds -Q1; the qacc combine
            # uses subtract to restore the sign
            for n in range(NC_N):
                for j in range(J):
                    nc.tensor.matmul(Sp[:, 1, n, :], W2_sb[:, n, j, :],
                                     cols(un2_128, j, n), start=(j == 0),
                                     stop=(j == J - 1))

        # ---- squash helpers (column layout [o, pairs]) ----
        def squash_pre(sb, final=False):
            """sb -> coef [1,PAIRS]. Engines: Act+DVE+2 PE mms."""
            s2 = sq_pool.tile([P, PAIRS], BF16, tag="s2")
            nc.vector.tensor_tensor(s2[:, :], sb[:, :], sb[:, :], op=mult)
            gp = paux.tile([P, 4 * P], BF16, tag="aux")
            g = gp[:, :].bitcast(F32)[0:1, 0:PAIRS]
            nc.tensor.matmul(g, ones_colb[:, :], s2[:, :], start=True,
                             stop=False)
            nc.tensor.matmul(g, ones_rowb[0:1, 0:1], ones_rowb[0:1, 0:PAIRS],
                             start=False, stop=True)
            with nc.allow_low_precision(reason="coef feeds 2e-2-gate output"):
                rt = sq_pool.tile([1, PAIRS], F32 if final else BF16,
                                  tag="rt")
                nc.scalar.activation(rt[:, :], g,
                                     mybir.ActivationFunctionType.Sqrt,
                                     bias=neg1_col[0:1, 0:1])
                rg = sq_pool.tile([1, PAIRS], F32 if final else BF16,
                                  tag="rg")
                nc.vector.reciprocal(rg[:, :], g)
                coef = sq_pool.tile([1, PAIRS], F32 if final else BF16,
                                    tag="coef")
                nc.vector.tensor_tensor(coef[:, :], rt[:, :], rg[:, :],
                                        op=mult)
            return coef

        def coef_bcast(coef, final=False):
            cbp = paux.tile([P, 4 * P], BF16, tag="aux")
            coefb = cbp[:, :].bitcast(F32)[:, 0:PAIRS]
            ones = ones_row if final else ones_rowb
            nc.tensor.matmul(coefb, ones[:, :], coef[:, :],
                             start=True, stop=True)
            return coefb

        def build_sb1(final):
            """first sb: rOF*A0 (+bias folded) or region-2 + bias."""
            sb = sq_pool.tile([P, PAIRS], F32 if final else BF16, tag="sb")
            if uniform_c0:
                if fold_bias:
                    nc.vector.tensor_scalar_mul(sb[:, :], Sp[:, 0, :, :], rOF)
                else:
                    nc.vector.scalar_tensor_tensor(
                        sb[:, :], Sp[:, 0, :, :], kscale * rOF, bias_bc,
                        op0=mult, op1=add)
            else:
                nc.vector.tensor_tensor(sb[:, :], Sp[:, 2, :, :], bias_bc,
                                        op=add)
            return sb

        # ---- routing iterations ----
        sb = build_sb1(final=(routings == 1))
        sb1_keep = sb
        if routings > 1 and not (uniform_c0 and fold_bias):
            # general path: base term must be rOF*A0 + bias, not phase-1 sb
            sb1_keep = sq_pool.tile([P, PAIRS], BF16, tag="sbk")
            nc.vector.tensor_scalar_mul(sb1_keep[:, :], Sp[:, 0, :, :], rOF)
            if not fold_bias:
                nc.vector.tensor_tensor(sb1_keep[:, :], sb1_keep[:, :],
                                        bias_bc, op=add)
        powers_done = False
        for it in range(2, routings + 1):
            last = it == routings
            coef = squash_pre(sb)
            if not powers_done:
                powers_done = True
                nc.vector.tensor_tensor(u2b[:, :, :], u_sb[:, :, :],
                                        u_sb[:, :, :], op=mult)
                nc.vector.tensor_scalar_mul(u128[:, :, :], u_sb[:, :, :], rOF)
                nc.vector.tensor_scalar_mul(un2_128[:, :, :], u2b[:, :, :],
                                            -rOF)
            # raw W^T @ sb products (PE, overlaps the coef chain)
            Mps = []
            for n in range(NC_N):
                Mp = pM.tile([P, J, B], F32, tag="M")
                for j in range(J):
                    nc.tensor.matmul(Mp[:, j, :], WT_sb[:, n, j, :],
                                     ncols(sb, n), start=True, stop=True)
                Mps.append(Mp)
            coefb = coef_bcast(coef)
            if it == 2:
                q1_chains()
            # raw (unscaled) delta movings, straight off the Mp psums:
            # dm1 = -u^2/OF * Mp (one psum input), dm2 = u/OF * dm1
            def nview(t_ap, n):
                return bass.AP(t_ap.tensor, t_ap.offset + n * B,
                               [t_ap.ap[0], [PAIRS, J], [1, B]])

            for n in range(NC_N):
                nc.vector.tensor_tensor(nview(dm1[:, :, :], n),
                                        nview(un2_128[:, :, :], n),
                                        Mps[n][:, :, :], op=mult)
            for n in range(NC_N):
                nc.vector.tensor_tensor(nview(dm2[:, :, :], n),
                                        nview(u128[:, :, :], n),
                                        nview(dm1[:, :, :], n), op=mult)
            # unscaled delta sums (coef commutes through the contraction)
            Dp = pD.tile([P, 2, NC_N, B], F32, tag="D")
            for n in range(NC_N):
                for j in range(J):
                    nc.tensor.matmul(Dp[:, 0, n, :], W_sb[:, n, j, :],
                                     cols(dm1, j, n), start=(j == 0),
                                     stop=(j == J - 1))
            for n in range(NC_N):
                for j in range(J):
                    nc.tensor.matmul(Dp[:, 1, n, :], W2_sb[:, n, j, :],
                                     cols(dm2, j, n), start=(j == 0),
                                     stop=(j == J - 1))
            # coefb -> sbuf on Act; vt on DVE; V accumulation on Pool
            cb_bf = sq_pool.tile([P, PAIRS], BF16, tag="cbs")
            nc.vector.tensor_copy(cb_bf[:, :], coefb)
            # scaled-increment accumulators (tiny [128,64] ops):
            #   pacc = sb0 + rOF * sum_t cb_t*D_A_t
            #   qacc = B0  +       sum_t cb_t*D_B_t
            iA = sq_pool.tile([P, PAIRS], BF16, tag="iA")
            nc.vector.tensor_tensor(iA[:, :], cb_bf[:, :], Dp[:, 0, :, :],
                                    op=mult)
            iB = sq_pool.tile([P, PAIRS], BF16, tag="iB")
            nc.vector.tensor_tensor(iB[:, :], cb_bf[:, :], Dp[:, 1, :, :],
                                    op=mult)
            vt = sq_pool.tile([P, PAIRS], BF16, tag="vt")
            nc.gpsimd.tensor_tensor(vt[:, :], sb[:, :], cb_bf[:, :], op=mult)
            if it == 2:
                nc.gpsimd.tensor_copy(V[:, :], vt[:, :])
            else:
                nc.gpsimd.tensor_tensor(V[:, :], V[:, :], vt[:, :], op=add)
            if it == 2:
                nc.vector.tensor_copy(sb0[:, :], sb1_keep[:, :])
            nc.vector.scalar_tensor_tensor(
                pacc[:, :], iA[:, :], rOF, sb0[:, :] if it == 2 else pacc[:, :],
                op0=mult, op1=add)
            if it == 2:
                nc.vector.tensor_tensor(qacc[:, :], iB[:, :], Sp[:, 1, :, :],
                                        op=mybir.AluOpType.subtract)
            else:
                nc.vector.tensor_tensor(qacc[:, :], qacc[:, :], iB[:, :],
                                        op=add)
            # sb_{t+1} = pacc + V*qacc
            e = sq_pool.tile([P, PAIRS], BF16, tag="e")
            nc.vector.tensor_tensor(e[:, :], V[:, :], qacc[:, :], op=mult)
            sb = sq_pool.tile([P, PAIRS], F32 if last else BF16, tag="sb")
            nc.vector.tensor_tensor(sb[:, :], pacc[:, :], e[:, :], op=add)

        # ---- final squash + output ----
        coef = squash_pre(sb, final=True)
        # transpose sb during the coef chain, then apply the per-pair coef
        # as a per-partition scalar on the row layout (n-major rows)
        # permute pairs to b-major with scatter-copies (cheap, off-path),
        # transpose early, then one contiguous output DMA
        sbm = sq_pool.tile([P, PAIRS], F32, tag="sbm")
        sb_ap2 = sb[:, :]
        sbm_ap = sbm[:, :]
        nc.scalar.copy(
            bass.AP(sbm_ap.tensor, sbm_ap.offset,
                    [sbm_ap.ap[0], [1, NC_N], [NC_N, B]]),
            sb_ap2)
        trvt = ptr.tile([P, 8 * P], BF16, tag="tr")
        trv = trvt[:, :].bitcast(F32)
        nc.tensor.transpose(trv[0:PAIRS, 0:OUT_F], sbm[:, :], identf[:, :])
        nc.scalar.copy(srows[:, :], trv[0:PAIRS, 0:OUT_F])
        cfb = sq_pool.tile([1, PAIRS], F32, tag="cfb")
        c_ap = coef[:, :]
        cfb_ap = cfb[:, :]
        nc.vector.tensor_copy(
            bass.AP(cfb_ap.tensor, cfb_ap.offset,
                    [cfb_ap.ap[0], [1, NC_N], [NC_N, B]]),
            c_ap)
        trc = paux.tile([P, 4 * P], BF16, tag="aux")
        trcf = trc[:, :].bitcast(F32)
        nc.tensor.transpose(trcf[0:PAIRS, 0:1], cfb[:, :], identf[0:1, 0:1])
        coefc = sq_pool.tile([PAIRS, 1], F32, tag="coefc")
        nc.vector.tensor_copy(coefc[:, :], trcf[0:PAIRS, 0:1])
        nc.vector.tensor_scalar_mul(vrows[:, :], srows[:, :], coefc[:, 0:1])
        nc.sync.dma_start(out_dram.ap().rearrange("b n o -> (b n) o"),
                          vrows[:, :])

    nc.compile()
    return nc


_NC_CACHE = {}


def _get_nc(key):
    if key not in _NC_CACHE:
        _NC_CACHE[key] = _build(*key)
    return _NC_CACHE[key]


def _pack_w(w, P, J, OUT_F):
    """[NC_N, IN_F, OUT_F] f32 -> [P, NC_N*J*OUT_F] bf16 in SBUF layout."""
    NC_N = w.shape[0]
    wl = w.reshape(NC_N, P, J, OUT_F).transpose(1, 0, 2, 3)
    return np.ascontiguousarray(wl.reshape(P, NC_N * J * OUT_F)).astype(
        ml_dtypes.bfloat16)


def _prep(u, weight, bias, c0, routings):
    u = np.asarray(u, dtype=np.float32)
    weight = np.asarray(weight, dtype=np.float32).reshape(weight.shape[-3:])
    bias = np.asarray(bias, dtype=np.float32).reshape(bias.shape[-2:])
    c0 = np.asarray(c0, dtype=np.float32).reshape(c0.shape[-2:])
    routings = int(routings)
    B, NUM, IN_F = u.shape
    OUT_F = weight.shape[-1]
    uniform = bool(np.all(c0 == c0.flat[0]))
    c00 = float(c0.flat[0])
    assert NUM % N_CORES == 0, f"NUM={NUM} not divisible by {N_CORES}"
    NC_N = NUM // N_CORES
    kscale = c00 * OUT_F if uniform else 1.0
    key = (NC_N, B, IN_F, OUT_F, routings, kscale, uniform)
    return u, weight, bias, c0, routings, NC_N, key, uniform


def _make_inmaps(u, weight, bias, c0, NC_N, uniform):
    B, NUM, IN_F = u.shape
    OUT_F = weight.shape[-1]
    P = 128
    J = IN_F // P
    ubf = u.astype(ml_dtypes.bfloat16)
    w2 = weight * weight
    wc = None if uniform else weight * c0[None]
    in_maps = []
    for c in range(N_CORES):
        sl = slice(c * NC_N, (c + 1) * NC_N)
        m = {
            "u": np.ascontiguousarray(ubf[:, sl, :]).reshape(B * NC_N, IN_F),
            "wbf": _pack_w(weight[sl], P, J, OUT_F),
            "w2bf": _pack_w(w2[sl], P, J, OUT_F),
            "biasc": np.ascontiguousarray(bias[sl].T),
            "biasr": np.ascontiguousarray(bias[sl].reshape(1, -1)),
        }
        if not uniform:
            m["wcbf"] = _pack_w(wc[sl], P, J, OUT_F)
        in_maps.append(m)
    return in_maps


def _gather(outs, B, NUM, OUT_F):
    # outs: list of [B, NC_N, OUT_F] per core -> [B, NUM, OUT_F]
    return np.concatenate(outs, axis=1)


def run_on_hw(u, weight, bias, c0, routings, trace=False):
    """Shard over cores, run SPMD, gather. Returns (out, exec_time_ns|None)."""
    u, weight, bias, c0, routings, NC_N, key, uniform = _prep(
        u, weight, bias, c0, routings)
    nc = _get_nc(key)
    in_maps = _make_inmaps(u, weight, bias, c0, NC_N, uniform)
    res = run_bass_kernel_spmd(nc, in_maps, core_ids=list(range(N_CORES)),
                               trace=trace)
    out = _gather([res.results[c]["out"] for c in range(N_CORES)],
                  u.shape[0], u.shape[1], weight.shape[-1])
    return out, res.exec_time_ns


_RUNNER_CACHE = {}


def _get_runner(key):
    """Cached jitted multi-core executable (avoids per-call re-jit)."""
    if key in _RUNNER_CACHE:
        return _RUNNER_CACHE[key]
    import jax
    from jax.sharding import Mesh, PartitionSpec
    from jax.experimental.shard_map import shard_map
    from concourse import bass2jax, mybir as mb

    nc = _get_nc(key)
    bass2jax.install_neuronx_cc_hook()
    part_name = nc.partition_id_tensor.name if nc.partition_id_tensor else None
    in_names, out_names, out_avals, zero_outs = [], [], [], []
    for alloc in nc.m.functions[0].allocations:
        if not isinstance(alloc, mb.MemoryLocationSet):
            continue
        name = alloc.memorylocations[0].name
        if alloc.kind == "ExternalInput":
            if name != part_name:
                in_names.append(name)
        elif alloc.kind == "ExternalOutput":
            out_names.append(name)
            shape = tuple(alloc.tensor_shape)
            dtype = mb.dt.np(alloc.dtype)
            out_avals.append(jax.core.ShapedArray(shape, dtype))
            zero_outs.append(np.zeros(shape, dtype))
    n_params = len(in_names)
    all_names = in_names + out_names
    if part_name is not None:
        all_names = all_names + [part_name]
    donate = tuple(range(n_params, n_params + len(out_names)))

    def _body(*args):
        operands = list(args)
        if part_name is not None:
            operands.append(bass2jax.partition_id_tensor())
        outs = bass2jax._bass_exec_p.bind(
            *operands,
            out_avals=tuple(out_avals),
            in_names=tuple(all_names),
            out_names=tuple(out_names),
            lowering_input_output_aliases=(),
            sim_require_finite=True,
            sim_require_nnan=True,
            nc=nc,
        )
        return tuple(outs)

    devices = jax.devices()[:N_CORES]
    mesh = Mesh(np.asarray(devices), ("core",))
    specs = (PartitionSpec("core"),) * (n_params + len(out_names))
    fn = jax.jit(
        shard_map(
            _body,
            mesh=mesh,
            in_specs=specs,
            out_specs=(PartitionSpec("core"),) * len(out_names),
            check_rep=False,
        ),
        donate_argnums=donate,
        keep_unused=True,
    )
    runner = (fn, in_names, out_names, out_avals, zero_outs)
    _RUNNER_CACHE[key] = runner
    return runner


def run_cached(u, weight, bias, c0, routings):
    """Run via a cached jitted executable. Returns (out, per_call_fn)."""
    u, weight, bias, c0, routings, NC_N, key, uniform = _prep(
        u, weight, bias, c0, routings)
    fn, in_names, out_names, out_avals, zero_outs = _get_runner(key)
    in_maps = _make_inmaps(u, weight, bias, c0, NC_N, uniform)
    concat_in = [
        np.concatenate([m[nm] for m in in_maps], axis=0) for nm in in_names
    ]
    B, NUM = u.shape[0], u.shape[1]
    OUT_F = weight.shape[-1]

    def call():
        zeros = [
            np.zeros((N_CORES * z.shape[0], *z.shape[1:]), z.dtype)
            for z in zero_outs
        ]
        outs = fn(*concat_in, *zeros)
        return np.asarray(outs[out_names.index("out")])

    full = call()
    per_core = full.reshape(N_CORES, B, NC_N, OUT_F)
    out = _gather(list(per_core), B, NUM, OUT_F)
    return out, call


def kernel(**inputs):
    out, _ = run_cached(
        inputs["u"],
        inputs["weight"],
        inputs["bias"],
        inputs["c0"],
        inputs["routings"],
    )
    return out
